# revision 16
# baseline (speedup 1.0000x reference)
"""Single-head attention (B=4, S=4096, D=1024, H=128, fp32) on 8 TRN2 NeuronCores.

Sharding: core i handles batch b = i//2, query-half h = i%2 (2048 queries).
KV for the full sequence is computed on-core from the full x[b] (duplicated
across the pair; collectives hang under this runtime, and the duplicate KV
projection costs ~15% extra PE time). The host passes x[b]^T with the
sequence axis rotated so the core's own query half always sits in columns
0:2048 — every core then runs an identical SPMD graph, and attention is
invariant to the key/value ordering.

Compute layout (per core):
  qT,kT [H,S] fp32r and vT->v [S,H] bf16 via PE-transpose.
  scoresT[sk,sq] = kT_chunk.T @ qT  (fp32r matmul, PSUM fp32)
  exp on ScalarE fused with the 1/sqrt(H) scale, PSUM->SBUF bf16.
  out^T[h,sq] += v_chunk.T @ expT   (bf16 matmul, PSUM accumulate)
  softmax denominators: DVE accumulates expT tiles over sk-chunks, then a
  PE transpose + free-axis reduce gives per-query sums; the final epilogue
  transposes out^T back and multiplies by the reciprocal.
"""

import numpy as np
import ml_dtypes

from concourse import bacc, mybir, tile, masks
from concourse.bass_utils import run_bass_kernel_spmd

B, S, D, H = 4, 4096, 1024, 128
SQ = S // 2            # queries per core
N_CORES = 8
SCALE = 1.0 / float(np.sqrt(np.float32(H)))

F32 = mybir.dt.float32
F32R = mybir.dt.float32r
BF16 = mybir.dt.bfloat16

DC = D // 128          # 8 contraction chunks
NSC = S // 512         # 8 s-chunks (full seq)
NQC = SQ // 512        # 4 query chunks per core
NSK = S // 128         # 32 key chunks
MAC = 2                # sk-chunks per exp macro (1024-wide ACT reads)


def build():
    nc = bacc.Bacc("TRN2", target_bir_lowering=False, debug=False,
                   num_devices=N_CORES)
    xt_ext = nc.dram_tensor("xt", [D, S], BF16, kind="ExternalInput").ap()
    w_ext = {k: nc.dram_tensor(f"w{k}", [D, H], BF16, kind="ExternalInput").ap()
             for k in "qkv"}
    b_ext = {k: nc.dram_tensor(f"b{k}", [H], F32, kind="ExternalInput").ap()
             for k in "qkv"}
    out_ext = nc.dram_tensor("out", [SQ, H], F32, kind="ExternalOutput").ap()

    with tile.TileContext(nc) as tc:
        with (
            tc.tile_pool(name="const", bufs=1) as constp,
            tc.tile_pool(name="big", bufs=1) as big,
            tc.tile_pool(name="work", bufs=3) as work,
            tc.tile_pool(name="accp", bufs=2) as accp,
            tc.tile_pool(name="outp", bufs=2) as outp,
            tc.tile_pool(name="scal", bufs=8) as scal,
        ):
            ident_bf = constp.tile([128, 128], BF16, tag="ident_bf")
            ident_f32 = constp.tile([128, 128], F32, tag="ident_f32")
            masks.make_identity(nc, ident_bf[:])
            masks.make_identity(nc, ident_f32[:])

            # ---- loads (weights/biases first: first matmuls need them) ----
            wsb = {}
            bsb = {}
            for k in "qkv":
                wsb[k] = constp.tile([128, DC * H], BF16, tag=f"w{k}", name=f"w{k}sb")
                nc.sync.dma_start(
                    wsb[k][:].rearrange("p (c h) -> p c h", c=DC),
                    w_ext[k].rearrange("(c p) h -> p c h", p=128))
                bsb[k] = constp.tile([128, 1], F32, tag=f"b{k}", name=f"b{k}sb")
                nc.sync.dma_start(bsb[k][:], b_ext[k][:])
            xt = [big.tile([128, S], BF16, tag=f"xt{c}", name=f"xt{c}")
                  for c in range(DC)]
            XSPLIT = 4
            w0 = S // XSPLIT
            for c in range(DC):
                for s in range(XSPLIT):
                    nc.sync.dma_start(
                        xt[c][:, s * w0:(s + 1) * w0],
                        xt_ext[c * 128:(c + 1) * 128, s * w0:(s + 1) * w0])

            # ---- projections (weight-stationary, kv interleaved per xt tile) ----
            kt = big.tile([128, S], F32R, tag="kt")
            vt = big.tile([128, S], BF16, tag="vt")
            qt = big.tile([128, SQ], F32R, tag="qt")
            pproj_cm = tc.tile_pool(name="pproj", bufs=1, space="PSUM")
            pproj = pproj_cm.__enter__()

            GRP = 4
            for g in range(NSC // GRP):
                ppk = [pproj.tile([128, 512], F32, tag=f"ppk{i}", name=f"ppk{i}")
                       for i in range(GRP)]
                ppv = [pproj.tile([128, 512], F32, tag=f"ppv{i}", name=f"ppv{i}")
                       for i in range(GRP)]
                for c in range(DC):
                    for i in range(GRP):
                        sc = g * GRP + i
                        nc.tensor.matmul(
                            ppk[i][:], wsb["k"][:, c * H:(c + 1) * H],
                            xt[c][:, sc * 512:(sc + 1) * 512],
                            start=(c == 0), stop=(c == DC - 1))
                    for i in range(GRP):
                        sc = g * GRP + i
                        nc.tensor.matmul(
                            ppv[i][:], wsb["v"][:, c * H:(c + 1) * H],
                            xt[c][:, sc * 512:(sc + 1) * 512],
                            start=(c == 0), stop=(c == DC - 1))
                for i in range(GRP):
                    sc = g * GRP + i
                    nc.vector.tensor_scalar_add(
                        kt[:, sc * 512:(sc + 1) * 512], ppk[i][:], bsb["k"][:])
                    nc.vector.tensor_scalar_add(
                        vt[:, sc * 512:(sc + 1) * 512], ppv[i][:], bsb["v"][:])
            for g in range(NQC // GRP):
                ppq = [pproj.tile([128, 512], F32, tag=f"ppk{i}", name=f"ppq{i}")
                       for i in range(GRP)]
                for c in range(DC):
                    for i in range(GRP):
                        sc = g * GRP + i
                        nc.tensor.matmul(
                            ppq[i][:], wsb["q"][:, c * H:(c + 1) * H],
                            xt[c][:, sc * 512:(sc + 1) * 512],
                            start=(c == 0), stop=(c == DC - 1))
                for i in range(GRP):
                    sc = g * GRP + i
                    nc.vector.tensor_scalar_add(
                        qt[:, sc * 512:(sc + 1) * 512], ppq[i][:], bsb["q"][:])

            # ---- v: [H,S] -> [S,H] tiles via DMA xbar transpose (bf16) ----
            v = big.tile([128, NSK * H], BF16, tag="v")
            for c in range(NSK):
                nc.sync.dma_start_transpose(
                    v[:, c * H:(c + 1) * H], vt[:, c * 128:(c + 1) * 128])
            pproj_cm.__exit__(None, None, None)

            # ---- attention ----
            att_cm = [tc.tile_pool(name="ps", bufs=3, space="PSUM"),
                      tc.tile_pool(name="po", bufs=1, space="PSUM"),
                      tc.tile_pool(name="ptmp", bufs=1, space="PSUM")]
            psco, pout, ptmp = [cm.__enter__() for cm in att_cm]
            n_mac = NSK // MAC
            for qc in range(NQC):
                qsl = qt[:, qc * 512:(qc + 1) * 512]
                po = pout.tile([128, 512], F32, tag="po")
                acc = accp.tile([128, MAC * 512], BF16, tag="acc")
                accg = accp.tile([128, MAC * 512], BF16, tag="accg")
                for m in range(n_mac):
                    ps = psco.tile([128, MAC * 512], F32, tag="ps")
                    for j in range(MAC):
                        sk = m * MAC + j
                        nc.tensor.matmul(
                            ps[:, j * 512:(j + 1) * 512],
                            kt[:, sk * 128:(sk + 1) * 128],
                            qsl, start=True, stop=True)
                    ex = work.tile([128, MAC * 512], BF16, tag="ex")
                    nc.scalar.activation(
                        ex[:], ps[:], mybir.ActivationFunctionType.Exp,
                        scale=SCALE)
                    for j in range(MAC):
                        sk = m * MAC + j
                        nc.tensor.matmul(
                            po[:],
                            v[:, sk * H:(sk + 1) * H],
                            ex[:, j * 512:(j + 1) * 512],
                            start=(m == 0 and j == 0),
                            stop=(m == n_mac - 1 and j == MAC - 1))
                    if m == 0:
                        nc.vector.tensor_copy(acc[:], ex[:])
                    elif m == 1:
                        nc.gpsimd.tensor_copy(accg[:], ex[:])
                    elif m % 2 == 0:
                        nc.vector.tensor_add(acc[:], acc[:], ex[:])
                    else:
                        nc.gpsimd.tensor_add(accg[:], accg[:], ex[:])

                # denominators: fold both accs to 512 wide, transpose, reduce
                acc5 = accp.tile([128, 512], BF16, tag="acc5")
                acc5g = accp.tile([128, 512], BF16, tag="acc5g")
                nc.vector.tensor_add(acc5[:], acc[:, :512], acc[:, 512:])
                nc.gpsimd.tensor_add(acc5g[:], accg[:, :512], accg[:, 512:])
                nc.vector.tensor_add(acc5[:], acc5[:], acc5g[:])
                pacc_h = ptmp.tile([128, 512], F32, tag="pno", name="pacc_h")
                pacc = pacc_h[:].bitcast(BF16)[:, :512]
                recs = []
                for t in range(4):
                    nc.tensor.transpose(
                        pacc[:, t * 128:(t + 1) * 128],
                        acc5[:, t * 128:(t + 1) * 128], ident_bf[:])
                for t in range(4):
                    den = scal.tile([128, 1], F32, tag="den")
                    nc.vector.reduce_sum(
                        den[:], pacc[:, t * 128:(t + 1) * 128],
                        axis=mybir.AxisListType.X)
                    rec = scal.tile([128, 1], F32, tag="rec")
                    nc.vector.reciprocal(rec[:], den[:])
                    recs.append(rec)

                # epilogue: normalize + transpose back to [sq, H]
                outc = outp.tile([128, 512], F32, tag="outc")
                nc.vector.tensor_copy(outc[:], po[:])
                pno = ptmp.tile([128, 512], F32, tag="pno")
                for t in range(4):
                    nc.tensor.transpose(
                        pno[:, t * 128:(t + 1) * 128],
                        outc[:, t * 128:(t + 1) * 128], ident_f32[:])
                ost = outp.tile([128, 512], F32, tag="ost")
                for t in range(4):
                    nc.vector.tensor_scalar_mul(
                        ost[:, t * 128:(t + 1) * 128],
                        pno[:, t * 128:(t + 1) * 128], recs[t][:])
                nc.sync.dma_start(
                    out_ext[qc * 512:(qc + 1) * 512, :]
                    .rearrange("(t p) h -> p t h", p=128),
                    ost[:].rearrange("p (t h) -> p t h", t=4))
            for cm in reversed(att_cm):
                cm.__exit__(None, None, None)
    nc.compile()
    return nc


_NC = None


def _get_nc():
    global _NC
    if _NC is None:
        _NC = build()
    return _NC


def make_in_maps(x, Wq, bq, Wk, bk, Wv, bv):
    x = np.asarray(x, dtype=np.float32)
    wq = np.asarray(Wq, dtype=ml_dtypes.bfloat16)
    wk = np.asarray(Wk, dtype=ml_dtypes.bfloat16)
    wv = np.asarray(Wv, dtype=ml_dtypes.bfloat16)
    bq = np.asarray(bq, dtype=np.float32)
    bk = np.asarray(bk, dtype=np.float32)
    bv = np.asarray(bv, dtype=np.float32)
    in_maps = []
    for core in range(N_CORES):
        b, half = divmod(core, 2)
        xr = np.concatenate([x[b, half * SQ:], x[b, :half * SQ]], axis=0)
        xt = np.ascontiguousarray(xr.T).astype(ml_dtypes.bfloat16)
        in_maps.append({
            "xt": xt, "wq": wq, "bq": bq, "wk": wk, "bk": bk,
            "wv": wv, "bv": bv,
        })
    return in_maps


def run(in_maps, trace=False, **kw):
    nc = _get_nc()
    return run_bass_kernel_spmd(nc, in_maps, core_ids=list(range(N_CORES)),
                                trace=trace, **kw)


def kernel(x, Wq, bq, Wk, bk, Wv, bv):
    res = run(make_in_maps(x, Wq, bq, Wk, bk, Wv, bv))
    out = np.empty((B, S, H), dtype=np.float32)
    for core in range(N_CORES):
        b, half = divmod(core, 2)
        out[b, half * SQ:(half + 1) * SQ, :] = res.results[core]["out"]
    return out
